# revision 40
# baseline (speedup 1.0000x reference)
"""Trainium2 Bass kernel for nn_Attention_32126355374702.

Computation (per reference):
    q_proj = q @ W2 + b2                  # [H]
    k_proj = einsum("bsh,hd->bsd", k, W1) + b1
    hidden = tanh(k_proj + q_proj)        # [B,S,H]
    score  = einsum("bsh,ho->bso", hidden, V) + bv
    out    = softmax(score, axis=1)       # [B,S,1]

Sharding: data-parallel over batch across 8 cores (8 batches/core).
W1/W2/V/biases replicated. Softmax is per-(batch) over S -> no collectives.

Per-core layout strategy: compute hidden TRANSPOSED ([c, rows]) so that
  - W1 streams in natural [h, c] layout as the stationary matmul operand,
  - the (b1 + q@W2 + b2) bias is a per-partition scalar fused into the
    tanh activation,
  - the V contraction is a single M=1 matmul over partitions.
k rows are transposed on the PE (identity-matmul transpose-mode).
Matmuls run as float32r (full fp32 precision, 1 col/cycle at N>=512).
bv is dropped: softmax over S is invariant to a constant shift.
"""

import sys

if "/opt/trn_rl_repo" not in sys.path:
    sys.path.insert(0, "/opt/trn_rl_repo")

from contextlib import ExitStack

import numpy as np

import concourse.bass as bass
from concourse import bacc, mybir
from concourse.masks import make_identity
from concourse.tile import TileContext

H = 256          # hidden dim
B = 64           # full batch
S = 2048         # sequence length
NCORES = 8
B_LOC = B // NCORES          # batches per core
R = B_LOC * S                # rows per core
CHUNK = 512                  # rows per chunk
NCHUNK = R // CHUNK
F32 = mybir.dt.float32
F32R = mybir.dt.float32r

# float32r: TF32-like reduced-mantissa matmul input format; runs at 1
# col/cycle (vs 4 for fp32) when the moving dim is >= 256. The BIR verifier
# requires every operand of an fp32r matmul to be *produced* as fp32r
# (a rounding cast) — so the kT copy (DVE), the tanh (ACT) and the W1/V
# constant loads (gpsimd cast-DMA) emit float32r directly.
USE_F32R = True
MMDT = F32R if USE_F32R else F32
# bf16 for the score stage (hidden/V): enables FWL on the score weight
# loads (4x). Costs ~1e-3 extra relative error on the output.
SCORE_BF16 = False
SCDT = mybir.dt.bfloat16 if SCORE_BF16 else MMDT
# run the PE transposes in float32r (1.5 vs 2.0 cyc/row): k is declared
# as float32r in DRAM (same 4-byte payload; the PE's hi/lo split rounds
# on ingestion).
TR_F32R = True
KDT = F32R if TR_F32R else F32


def build_program(repeat: int = 1) -> bass.Bass:
    # Bacc.finalize() runs move_matmul_waits_to_ldweights +
    # generate_event_semaphores — required so self-loading fp32/fp32r
    # matmuls never carry more sync waits than the S3_LW slot allows.
    nc = bacc.Bacc(None, target_bir_lowering=False)

    q_d = nc.declare_dram_parameter("q", [H], F32, isOutput=False)
    k_d = nc.declare_dram_parameter("k", [B_LOC, S, H], KDT, isOutput=False)
    w1_d = nc.declare_dram_parameter("W1", [H, H], F32, isOutput=False)
    b1_d = nc.declare_dram_parameter("b1", [H], F32, isOutput=False)
    w2_d = nc.declare_dram_parameter("W2", [H, H], F32, isOutput=False)
    b2_d = nc.declare_dram_parameter("b2", [H], F32, isOutput=False)
    v_d = nc.declare_dram_parameter("V", [H, 1], F32, isOutput=False)
    out_d = nc.declare_dram_parameter("out", [B_LOC, S, 1], F32, isOutput=True)

    with TileContext(nc) as tc:
        with ExitStack() as ctx:
            _emit(nc, tc, ctx, q_d, k_d, w1_d, b1_d, w2_d, b2_d, v_d, out_d, repeat)
    # Run the Bacc passes (matmul wait-splitting, event semaphores, register
    # allocation) — the PJRT exec path serializes nc as-is and never calls
    # finalize itself.
    nc.finalize()
    return nc


def _emit(nc, tc, ctx, q_d, k_d, w1_d, b1_d, w2_d, b2_d, v_d, out_d, repeat=1):
    singles = ctx.enter_context(tc.tile_pool(name="singles", bufs=1))
    kin = ctx.enter_context(tc.tile_pool(name="kin", bufs=4))
    ktp = ctx.enter_context(tc.tile_pool(name="ktp", bufs=3))
    htp = ctx.enter_context(tc.tile_pool(name="htp", bufs=6))
    # PSUM budget (8 banks): psum_t = 2-bank tiles x2, psum_h = 1 bank x2,
    # psum_s = 1 bank x2
    psum_t = ctx.enter_context(tc.tile_pool(name="psum_t", bufs=2, space="PSUM"))
    psum_h = ctx.enter_context(tc.tile_pool(name="psum_h", bufs=2, space="PSUM"))
    psum_s = ctx.enter_context(tc.tile_pool(name="psum_s", bufs=2, space="PSUM"))

    Tanh = mybir.ActivationFunctionType.Tanh
    Exp = mybir.ActivationFunctionType.Exp
    Add = mybir.AluOpType.add
    X = mybir.AxisListType.X

    # ---------------- preamble: constants ----------------
    ident = singles.tile([128, 128], F32)
    make_identity(nc, ident)
    ident_r = singles.tile([128, 128], KDT)
    nc.vector.tensor_copy(out=ident_r, in_=ident)
    ones_col = singles.tile([128, 1], F32)
    nc.vector.memset(ones_col, 1.0)

    # W1 in natural [h, c] layout: W1_sb[p, t, c] = W1[t*128+p, c]
    # gpsimd DMA casts fp32 -> fp32r (rounding) for the fp32r matmuls.
    w1_sb = singles.tile([128, 2, H], MMDT)
    nc.gpsimd.dma_start(out=w1_sb, in_=w1_d[:].rearrange("(t p) c -> p t c", p=128))
    # all preamble loads go through SWDGE (gpsimd) so the SP queue starts
    # streaming k immediately
    w2_sb = singles.tile([128, 2, H], F32)
    nc.gpsimd.dma_start(out=w2_sb, in_=w2_d[:].rearrange("(t p) c -> p t c", p=128))

    # column layouts: x_col[p, t] = x[t*128 + p]
    q_sb = singles.tile([128, 2], F32)
    nc.gpsimd.dma_start(out=q_sb, in_=q_d[:].rearrange("(t p) -> p t", p=128))
    b1c = singles.tile([128, 2], F32)
    nc.gpsimd.dma_start(out=b1c, in_=b1_d[:].rearrange("(t p) -> p t", p=128))
    b2c = singles.tile([128, 2], F32)
    nc.gpsimd.dma_start(out=b2c, in_=b2_d[:].rearrange("(t p) -> p t", p=128))
    # V in column layout, replicated x2 along free so the fp32r score
    # matmuls have an even moving free-dim (ISA restriction). DMA can't
    # broadcast (zero-stride innermost), so replicate with a DVE copy.
    v_tmp = singles.tile([128, 2], SCDT)
    nc.gpsimd.dma_start(out=v_tmp, in_=v_d[:].rearrange("(t p) o -> p (t o)", p=128))
    v_sb = singles.tile([128, 2, 2], SCDT)
    nc.vector.tensor_copy(out=v_sb, in_=v_tmp.unsqueeze(2).broadcast_to([128, 2, 2]))

    # bias_c[p, j] = (q @ W2)[j*128+p] + b2[j*128+p] + b1[j*128+p]
    bias_c = singles.tile([128, 2], F32)
    for j in range(2):
        pb = psum_h.tile([128, 1], F32, tag="h", name=f"pb{j}")
        for t in range(2):
            nc.tensor.matmul(
                pb,
                lhsT=w2_sb[:, t, 128 * j : 128 * (j + 1)],
                rhs=q_sb[:, t : t + 1],
                start=(t == 0),
                stop=(t == 1),
            )
        nc.vector.tensor_tensor(
            out=bias_c[:, j : j + 1], in0=pb, in1=b1c[:, j : j + 1], op=Add
        )
        nc.vector.tensor_tensor(
            out=bias_c[:, j : j + 1],
            in0=bias_c[:, j : j + 1],
            in1=b2c[:, j : j + 1],
            op=Add,
        )

    # exp(score) assembled as [128, 128]: exp_all[p, 16*b + 4*c4 + g] =
    # exp(score[b, 512*c4 + 128*g + p])  (b = batch, c4 = chunk-in-batch,
    # g = 128-row group in chunk). Written directly by the per-chunk exp.
    exp_all = singles.tile([128, 128], F32)

    kflat = k_d[:].rearrange("b s h -> (b s) h")

    # ---------------- main loop (software-pipelined by one chunk) ----------
    # Per-chunk stages:
    #   load(ci):  DMA k chunk
    #   tmm(ci):   PE transposes + DVE copies + main matmuls -> psum
    #   act(ci):   ACT tanh -> hidden tiles
    #   score(ci): PE V-matmuls -> psum score row
    #   fin(ci):   ACT exp + scatter-DMA into exp_all
    # Emission order interleaves score(ci-1) between the transposes and the
    # main matmuls of chunk ci so the PE stream never waits on ACT's tanh.
    def stage_load(pair, split=False):
        # one 1 MiB DMA covers two 512-row chunks; the first pair is split
        # in two so the pipeline fills sooner
        r0 = pair * 2 * CHUNK
        k_nat = kin.tile([128, 8, H], KDT, tag="k", name=f"knat{pair}")
        if split:
            for half in range(2):
                nc.sync.dma_start(
                    out=k_nat[:, 4 * half : 4 * half + 4, :],
                    in_=kflat[
                        r0 + half * CHUNK : r0 + (half + 1) * CHUNK, :
                    ].rearrange("(rg p) h -> p rg h", p=128),
                )
        else:
            nc.sync.dma_start(
                out=k_nat,
                in_=kflat[r0 : r0 + 2 * CHUNK, :].rearrange(
                    "(rg p) h -> p rg h", p=128
                ),
            )
        return k_nat

    def stage_transpose(ci, k_nat):
        # all 8 transposes of the chunk land in one 2-bank PSUM tile;
        # a single wide DVE copy (with the fp32 -> fp32r rounding cast)
        # moves them to SBUF.
        rg0 = 4 * (ci % 2)
        pt = psum_t.tile([128, 2, 4, 128], KDT, tag="t", name=f"pt{ci}")
        kt = ktp.tile([128, 2, 4, 128], MMDT, tag="kt", name=f"kt{ci}")
        for t in range(2):
            for rg in range(4):
                nc.tensor.transpose(
                    out=pt[:, t, rg, :],
                    in_=k_nat[:, rg0 + rg, 128 * t : 128 * (t + 1)],
                    identity=ident_r if TR_F32R else ident,
                )
            # per-h-tile copy so main matmuls can start on kt[t=0] while
            # the t=1 transposes are still draining
            nc.vector.tensor_copy(out=kt[:, t], in_=pt[:, t])
        return [kt[:, t].rearrange("p a c -> p (a c)") for t in range(2)]

    def stage_main(ci, kts):
        phs = []
        for j in range(2):
            ph = psum_h.tile([128, CHUNK], F32, tag="h", name=f"ph{ci}_{j}")
            for t in range(2):
                nc.tensor.matmul(
                    ph,
                    lhsT=w1_sb[:, t, 128 * j : 128 * (j + 1)],
                    rhs=kts[t],
                    start=(t == 0),
                    stop=(t == 1),
                )
            phs.append(ph)
        return phs

    def stage_tanh(ci, phs):
        hts = []
        for j in range(2):
            # ACT writes fp32r (rounding cast) so the score matmul can be fp32r
            ht = htp.tile([128, CHUNK], SCDT, tag="ht", name=f"ht{ci}_{j}")
            nc.scalar.activation(
                out=ht, in_=phs[j], func=Tanh, bias=bias_c[:, j : j + 1], scale=1.0
            )
            hts.append(ht)
        return hts

    def stage_score(ci, hts):
        # transposed score: out[p, 2g:2g+2] = sum_c hT[c, 128g + p] * V[c]
        # (pair-duplicated along free for the fp32r even-free-dim ISA rule).
        # lhsT = hidden slice (M=128 rows), rhs = V pair (N=2) -> the matmul
        # stream is 2 columns; the hT weight loads hide under the
        # main-matmul streams.
        ps = psum_s.tile([128, 4, 2], F32, tag="s", name=f"ps{ci}")
        for g in range(4):
            for j in range(2):
                nc.tensor.matmul(
                    ps[:, g, :],
                    lhsT=hts[j][:, 128 * g : 128 * (g + 1)],
                    rhs=v_sb[:, j, :],
                    start=(j == 0),
                    stop=(j == 1),
                )
        return ps

    def stage_fin(ci, ps):
        # 128-lane exp straight from PSUM (strided over the pairs) into the
        # assembled layout
        c0 = 4 * ci
        nc.scalar.activation(out=exp_all[:, c0 : c0 + 4], in_=ps[:, :, 0], func=Exp)

    for rep in range(repeat):
        # per-(p, batch) exp partials, filled incrementally as batches finish
        sums8 = singles.tile([128, 8], F32, name=f"sums8_{rep}", tag="sums8")

        k_pairs = {0: stage_load(0, split=True)}
        pending = []  # [(ci, hts)] awaiting score, two-chunk skew

        def flush_one():
            p0, hts = pending.pop(0)
            stage_fin(p0, stage_score(p0, hts))
            # batch sums, a few chunks behind so the DVE queue never
            # stalls waiting for a just-issued exp
            done = p0 - 2
            if done >= 0 and done % 4 == 3:
                b = done // 4
                nc.vector.tensor_reduce(
                    out=sums8[:, b : b + 1],
                    in_=exp_all[:, 16 * b : 16 * (b + 1)],
                    axis=X,
                    op=Add,
                )

        for ci in range(NCHUNK):
            pair = ci // 2
            if ci % 2 == 0 and pair + 1 < NCHUNK // 2:
                k_pairs[pair + 1] = stage_load(pair + 1)
            kts = stage_transpose(ci, k_pairs[pair])
            if ci % 2 == 1:
                del k_pairs[pair]
            if len(pending) >= 2:
                flush_one()
            phs = stage_main(ci, kts)
            hts = stage_tanh(ci, phs)
            pending.append((ci, hts))
        while pending:
            flush_one()
        nc.vector.tensor_reduce(
            out=sums8[:, B_LOC - 1 : B_LOC],
            in_=exp_all[:, 16 * (B_LOC - 1) : 16 * B_LOC],
            axis=X,
            op=Add,
        )

        # ---------------- softmax epilogue ----------------
        # partition-sum of sums8 in one matmul with a ones vector
        psb = psum_s.tile([1, 8], F32, tag="s", name=f"psb{rep}")
        nc.tensor.matmul(psb, lhsT=ones_col, rhs=sums8, start=True, stop=True)
        brec = singles.tile([1, 8], F32, name=f"brec{rep}", tag="brec")
        nc.vector.reciprocal(out=brec, in_=psb)
        recip_bc = singles.tile([128, 8], F32, name=f"recip_bc{rep}", tag="recip_bc")
        nc.gpsimd.partition_broadcast(recip_bc, brec)

        # attn[p, b, i] = exp_all[p, b, i] * recip_bc[p, b]
        attn = singles.tile([128, 128], F32, name=f"attn{rep}", tag="attn")
        nc.vector.tensor_tensor(
            out=attn.rearrange("p (b i) -> p b i", i=16),
            in0=exp_all.rearrange("p (b i) -> p b i", i=16),
            in1=recip_bc.unsqueeze(2).broadcast_to([128, 8, 16]),
            op=mybir.AluOpType.mult,
        )

        # transpose so the output DMA writes 512B-contiguous runs:
        # attn_t[col, p] with col = (b, c4, g), s = 512*c4 + 128*g + p
        ptr3 = psum_t.tile([128, 128], F32, tag="t", name=f"ptr3_{rep}")
        nc.tensor.transpose(out=ptr3, in_=attn, identity=ident)
        attn_t = singles.tile([128, 128], F32, name=f"attn_t{rep}", tag="attn_t")
        nc.scalar.copy(out=attn_t, in_=ptr3)

        nc.sync.dma_start(
            out=out_d[:].rearrange("b (c4 g p) o -> (b c4 g) (p o)", c4=4, g=4),
            in_=attn_t,
        )


_program_cache = {}


def _get_program() -> bass.Bass:
    if "nc" not in _program_cache:
        _program_cache["nc"] = build_program()
    return _program_cache["nc"]


def kernel(q, k, W1, b1, W2, b2, V, bv=None, **_unused):
    from concourse.bass_utils import run_bass_kernel_spmd

    q = np.ascontiguousarray(np.asarray(q, dtype=np.float32))
    k = np.ascontiguousarray(np.asarray(k, dtype=np.float32))
    W1 = np.ascontiguousarray(np.asarray(W1, dtype=np.float32))
    b1 = np.ascontiguousarray(np.asarray(b1, dtype=np.float32))
    W2 = np.ascontiguousarray(np.asarray(W2, dtype=np.float32))
    b2 = np.ascontiguousarray(np.asarray(b2, dtype=np.float32))
    V = np.ascontiguousarray(np.asarray(V, dtype=np.float32))

    nc = _get_program()
    in_maps = []
    for c in range(NCORES):
        in_maps.append(
            {
                "q": q,
                "k": np.ascontiguousarray(k[c * B_LOC : (c + 1) * B_LOC]),
                "W1": W1,
                "b1": b1,
                "W2": W2,
                "b2": b2,
                "V": V,
            }
        )
    res = run_bass_kernel_spmd(nc, in_maps, list(range(NCORES)))
    outs = [res.results[c]["out"] for c in range(NCORES)]
    return np.concatenate(outs, axis=0).astype(np.float32)


if __name__ == "__main__":
    # smoke: build only
    nc = build_program()
    print("built ok")


# revision 41
# speedup vs baseline: 1662.8329x; 1662.8329x over previous
"""Trainium2 Bass kernel for nn_Attention_32126355374702.

Computation (per reference):
    q_proj = q @ W2 + b2                  # [H]
    k_proj = einsum("bsh,hd->bsd", k, W1) + b1
    hidden = tanh(k_proj + q_proj)        # [B,S,H]
    score  = einsum("bsh,ho->bso", hidden, V) + bv
    out    = softmax(score, axis=1)       # [B,S,1]

Sharding: data-parallel over batch across 8 cores (8 batches/core).
W1/W2/V/biases replicated. Softmax is per-(batch) over S -> no collectives.

Per-core layout strategy: compute hidden TRANSPOSED ([c, rows]) so that
  - W1 streams in natural [h, c] layout as the stationary matmul operand,
  - the (b1 + q@W2 + b2) bias is a per-partition scalar fused into the
    tanh activation,
  - the V contraction is a single M=1 matmul over partitions.
k rows are transposed on the PE (identity-matmul transpose-mode).
Matmuls run as float32r (full fp32 precision, 1 col/cycle at N>=512).
bv is dropped: softmax over S is invariant to a constant shift.
"""

import sys

if "/opt/trn_rl_repo" not in sys.path:
    sys.path.insert(0, "/opt/trn_rl_repo")

from contextlib import ExitStack

import numpy as np

import concourse.bass as bass
from concourse import bacc, mybir
from concourse.masks import make_identity
from concourse.tile import TileContext

H = 256          # hidden dim
B = 64           # full batch
S = 2048         # sequence length
NCORES = 8
B_LOC = B // NCORES          # batches per core
R = B_LOC * S                # rows per core
CHUNK = 512                  # rows per chunk
NCHUNK = R // CHUNK
F32 = mybir.dt.float32
F32R = mybir.dt.float32r

# float32r: TF32-like reduced-mantissa matmul input format; runs at 1
# col/cycle (vs 4 for fp32) when the moving dim is >= 256. The BIR verifier
# requires every operand of an fp32r matmul to be *produced* as fp32r
# (a rounding cast) — so the kT copy (DVE), the tanh (ACT) and the W1/V
# constant loads (gpsimd cast-DMA) emit float32r directly.
USE_F32R = True
MMDT = F32R if USE_F32R else F32
# bf16 for the score stage (hidden/V): enables FWL on the score weight
# loads (4x). Costs ~1e-3 extra relative error on the output.
SCORE_BF16 = False
SCDT = mybir.dt.bfloat16 if SCORE_BF16 else MMDT
# run the PE transposes in float32r (1.5 vs 2.0 cyc/row): k is declared
# as float32r in DRAM (same 4-byte payload; the PE's hi/lo split rounds
# on ingestion).
TR_F32R = True
KDT = F32R if TR_F32R else F32


def build_program(repeat: int = 1) -> bass.Bass:
    # Bacc.finalize() runs move_matmul_waits_to_ldweights +
    # generate_event_semaphores — required so self-loading fp32/fp32r
    # matmuls never carry more sync waits than the S3_LW slot allows.
    nc = bacc.Bacc(None, target_bir_lowering=False)

    q_d = nc.declare_dram_parameter("q", [H], F32, isOutput=False)
    k_d = nc.declare_dram_parameter("k", [B_LOC, S, H], KDT, isOutput=False)
    w1_d = nc.declare_dram_parameter("W1", [H, H], F32, isOutput=False)
    b1_d = nc.declare_dram_parameter("b1", [H], F32, isOutput=False)
    w2_d = nc.declare_dram_parameter("W2", [H, H], F32, isOutput=False)
    b2_d = nc.declare_dram_parameter("b2", [H], F32, isOutput=False)
    v_d = nc.declare_dram_parameter("V", [H, 1], F32, isOutput=False)
    out_d = nc.declare_dram_parameter("out", [B_LOC, S, 1], F32, isOutput=True)

    with TileContext(nc) as tc:
        with ExitStack() as ctx:
            _emit(nc, tc, ctx, q_d, k_d, w1_d, b1_d, w2_d, b2_d, v_d, out_d, repeat)
    # Run the Bacc passes (matmul wait-splitting, event semaphores, register
    # allocation) — the PJRT exec path serializes nc as-is and never calls
    # finalize itself.
    nc.finalize()
    return nc


def _emit(nc, tc, ctx, q_d, k_d, w1_d, b1_d, w2_d, b2_d, v_d, out_d, repeat=1):
    singles = ctx.enter_context(tc.tile_pool(name="singles", bufs=1))
    kin = ctx.enter_context(tc.tile_pool(name="kin", bufs=4))
    ktp = ctx.enter_context(tc.tile_pool(name="ktp", bufs=3))
    htp = ctx.enter_context(tc.tile_pool(name="htp", bufs=6))
    # PSUM budget (8 banks): psum_t = 2-bank tiles x2, psum_h = 1 bank x2,
    # psum_s = 1 bank x2
    psum_t = ctx.enter_context(tc.tile_pool(name="psum_t", bufs=2, space="PSUM"))
    psum_h = ctx.enter_context(tc.tile_pool(name="psum_h", bufs=2, space="PSUM"))
    psum_s = ctx.enter_context(tc.tile_pool(name="psum_s", bufs=2, space="PSUM"))

    Tanh = mybir.ActivationFunctionType.Tanh
    Exp = mybir.ActivationFunctionType.Exp
    Add = mybir.AluOpType.add
    X = mybir.AxisListType.X

    # ---------------- preamble: constants ----------------
    ident = singles.tile([128, 128], F32)
    make_identity(nc, ident)
    ident_r = singles.tile([128, 128], KDT)
    nc.vector.tensor_copy(out=ident_r, in_=ident)
    ones_col = singles.tile([128, 1], F32)
    nc.vector.memset(ones_col, 1.0)

    # W1 in natural [h, c] layout: W1_sb[p, t, c] = W1[t*128+p, c]
    # gpsimd DMA casts fp32 -> fp32r (rounding) for the fp32r matmuls.
    w1_sb = singles.tile([128, 2, H], MMDT)
    nc.gpsimd.dma_start(out=w1_sb, in_=w1_d[:].rearrange("(t p) c -> p t c", p=128))
    # all preamble loads go through SWDGE (gpsimd) so the SP queue starts
    # streaming k immediately
    w2_sb = singles.tile([128, 2, H], F32)
    nc.gpsimd.dma_start(out=w2_sb, in_=w2_d[:].rearrange("(t p) c -> p t c", p=128))

    # column layouts: x_col[p, t] = x[t*128 + p]
    q_sb = singles.tile([128, 2], F32)
    nc.gpsimd.dma_start(out=q_sb, in_=q_d[:].rearrange("(t p) -> p t", p=128))
    b1c = singles.tile([128, 2], F32)
    nc.gpsimd.dma_start(out=b1c, in_=b1_d[:].rearrange("(t p) -> p t", p=128))
    b2c = singles.tile([128, 2], F32)
    nc.gpsimd.dma_start(out=b2c, in_=b2_d[:].rearrange("(t p) -> p t", p=128))
    # V in column layout, replicated x2 along free so the fp32r score
    # matmuls have an even moving free-dim (ISA restriction). DMA can't
    # broadcast (zero-stride innermost), so replicate with a DVE copy.
    v_tmp = singles.tile([128, 2], SCDT)
    nc.gpsimd.dma_start(out=v_tmp, in_=v_d[:].rearrange("(t p) o -> p (t o)", p=128))
    v_sb = singles.tile([128, 2, 2], SCDT)
    nc.vector.tensor_copy(out=v_sb, in_=v_tmp.unsqueeze(2).broadcast_to([128, 2, 2]))

    # bias_c[p, j] = (q @ W2)[j*128+p] + b2[j*128+p] + b1[j*128+p]
    bias_c = singles.tile([128, 2], F32)
    for j in range(2):
        pb = psum_h.tile([128, 1], F32, tag="h", name=f"pb{j}")
        for t in range(2):
            nc.tensor.matmul(
                pb,
                lhsT=w2_sb[:, t, 128 * j : 128 * (j + 1)],
                rhs=q_sb[:, t : t + 1],
                start=(t == 0),
                stop=(t == 1),
            )
        nc.vector.tensor_tensor(
            out=bias_c[:, j : j + 1], in0=pb, in1=b1c[:, j : j + 1], op=Add
        )
        nc.vector.tensor_tensor(
            out=bias_c[:, j : j + 1],
            in0=bias_c[:, j : j + 1],
            in1=b2c[:, j : j + 1],
            op=Add,
        )

    # exp(score) assembled as [128, 128]: exp_all[p, 16*b + 4*c4 + g] =
    # exp(score[b, 512*c4 + 128*g + p])  (b = batch, c4 = chunk-in-batch,
    # g = 128-row group in chunk). Written directly by the per-chunk exp.
    exp_all = singles.tile([128, 128], F32)

    kflat = k_d[:].rearrange("b s h -> (b s) h")

    # ---------------- main loop (software-pipelined by one chunk) ----------
    # Per-chunk stages:
    #   load(ci):  DMA k chunk
    #   tmm(ci):   PE transposes + DVE copies + main matmuls -> psum
    #   act(ci):   ACT tanh -> hidden tiles
    #   score(ci): PE V-matmuls -> psum score row
    #   fin(ci):   ACT exp + scatter-DMA into exp_all
    # Emission order interleaves score(ci-1) between the transposes and the
    # main matmuls of chunk ci so the PE stream never waits on ACT's tanh.
    def stage_load(pair, split=False):
        # one 1 MiB DMA covers two 512-row chunks; the first pair is split
        # in two so the pipeline fills sooner
        r0 = pair * 2 * CHUNK
        k_nat = kin.tile([128, 8, H], KDT, tag="k", name=f"knat{pair}")
        if split:
            for half in range(2):
                nc.sync.dma_start(
                    out=k_nat[:, 4 * half : 4 * half + 4, :],
                    in_=kflat[
                        r0 + half * CHUNK : r0 + (half + 1) * CHUNK, :
                    ].rearrange("(rg p) h -> p rg h", p=128),
                )
        else:
            nc.sync.dma_start(
                out=k_nat,
                in_=kflat[r0 : r0 + 2 * CHUNK, :].rearrange(
                    "(rg p) h -> p rg h", p=128
                ),
            )
        return k_nat

    def stage_transpose(ci, k_nat):
        # all 8 transposes of the chunk land in one 2-bank PSUM tile;
        # a single wide DVE copy (with the fp32 -> fp32r rounding cast)
        # moves them to SBUF.
        rg0 = 4 * (ci % 2)
        pt = psum_t.tile([128, 2, 4, 128], KDT, tag="t", name=f"pt{ci}")
        kt = ktp.tile([128, 2, 4, 128], MMDT, tag="kt", name=f"kt{ci}")
        for t in range(2):
            for rg in range(4):
                nc.tensor.transpose(
                    out=pt[:, t, rg, :],
                    in_=k_nat[:, rg0 + rg, 128 * t : 128 * (t + 1)],
                    identity=ident_r if TR_F32R else ident,
                )
            # per-h-tile copy so main matmuls can start on kt[t=0] while
            # the t=1 transposes are still draining
            nc.vector.tensor_copy(out=kt[:, t], in_=pt[:, t])
        return [kt[:, t].rearrange("p a c -> p (a c)") for t in range(2)]

    def stage_main(ci, kts):
        phs = []
        for j in range(2):
            ph = psum_h.tile([128, CHUNK], F32, tag="h", name=f"ph{ci}_{j}")
            for t in range(2):
                nc.tensor.matmul(
                    ph,
                    lhsT=w1_sb[:, t, 128 * j : 128 * (j + 1)],
                    rhs=kts[t],
                    start=(t == 0),
                    stop=(t == 1),
                )
            phs.append(ph)
        return phs

    def stage_tanh(ci, phs):
        hts = []
        for j in range(2):
            # ACT writes fp32r (rounding cast) so the score matmul can be fp32r
            ht = htp.tile([128, CHUNK], SCDT, tag="ht", name=f"ht{ci}_{j}")
            nc.scalar.activation(
                out=ht, in_=phs[j], func=Tanh, bias=bias_c[:, j : j + 1], scale=1.0
            )
            hts.append(ht)
        return hts

    def stage_score(ci, hts):
        # transposed score: out[p, 2g:2g+2] = sum_c hT[c, 128g + p] * V[c]
        # (pair-duplicated along free for the fp32r even-free-dim ISA rule).
        # lhsT = hidden slice (M=128 rows), rhs = V pair (N=2) -> the matmul
        # stream is 2 columns; the hT weight loads hide under the
        # main-matmul streams.
        ps = psum_s.tile([128, 4, 2], F32, tag="s", name=f"ps{ci}")
        for g in range(4):
            for j in range(2):
                nc.tensor.matmul(
                    ps[:, g, :],
                    lhsT=hts[j][:, 128 * g : 128 * (g + 1)],
                    rhs=v_sb[:, j, :],
                    start=(j == 0),
                    stop=(j == 1),
                )
        return ps

    def stage_fin(ci, ps):
        # 128-lane exp straight from PSUM (strided over the pairs) into the
        # assembled layout
        c0 = 4 * ci
        nc.scalar.activation(out=exp_all[:, c0 : c0 + 4], in_=ps[:, :, 0], func=Exp)

    for rep in range(repeat):
        # per-(p, batch) exp partials, filled incrementally as batches finish
        sums8 = singles.tile([128, 8], F32, name=f"sums8_{rep}", tag="sums8")

        def finalize_batches(b0, b1, tag):
            # softmax normalization + transposed store for batches [b0, b1)
            n = b1 - b0
            psb = psum_s.tile([1, n], F32, tag="s", name=f"psb{rep}_{tag}")
            nc.tensor.matmul(
                psb, lhsT=ones_col, rhs=sums8[:, b0:b1], start=True, stop=True
            )
            brec = singles.tile([1, n], F32, name=f"brec{rep}_{tag}", tag=f"br{tag}")
            nc.vector.reciprocal(out=brec, in_=psb)
            rbc = singles.tile([128, n], F32, name=f"rbc{rep}_{tag}", tag=f"rb{tag}")
            nc.gpsimd.partition_broadcast(rbc, brec)

            attn = singles.tile(
                [128, n, 16], F32, name=f"attn{rep}_{tag}", tag=f"at{tag}"
            )
            nc.vector.tensor_tensor(
                out=attn,
                in0=exp_all[:, 16 * b0 : 16 * b1].rearrange(
                    "p (b i) -> p b i", i=16
                ),
                in1=rbc.unsqueeze(2).broadcast_to([128, n, 16]),
                op=mybir.AluOpType.mult,
            )
            # transpose so the output DMA writes 512B-contiguous runs:
            # attn_t[col, p] with col = (b, c4, g), s = 512*c4 + 128*g + p
            ptr3 = psum_t.tile(
                [16 * n, 128], F32, tag="t", name=f"ptr3_{rep}_{tag}"
            )
            nc.tensor.transpose(
                out=ptr3, in_=attn.rearrange("p b i -> p (b i)"), identity=ident
            )
            attn_t = singles.tile(
                [16 * n, 128], F32, name=f"attn_t{rep}_{tag}", tag=f"att{tag}"
            )
            nc.scalar.copy(out=attn_t, in_=ptr3)
            nc.sync.dma_start(
                out=out_d[b0:b1].rearrange(
                    "b (c4 g p) o -> (b c4 g) (p o)", c4=4, g=4
                ),
                in_=attn_t,
            )

        k_pairs = {0: stage_load(0, split=True)}
        pending = []  # [(ci, hts)] awaiting score, two-chunk skew

        def flush_one():
            p0, hts = pending.pop(0)
            stage_fin(p0, stage_score(p0, hts))
            # batch sums, one chunk behind so the DVE queue never stalls
            # waiting for a just-issued exp
            done = p0 - 1
            if done >= 0 and done % 4 == 3:
                b = done // 4
                nc.vector.tensor_reduce(
                    out=sums8[:, b : b + 1],
                    in_=exp_all[:, 16 * b : 16 * (b + 1)],
                    axis=X,
                    op=Add,
                )

        for ci in range(NCHUNK):
            pair = ci // 2
            if ci % 2 == 0 and pair + 1 < NCHUNK // 2:
                k_pairs[pair + 1] = stage_load(pair + 1)
            kts = stage_transpose(ci, k_pairs[pair])
            if ci % 2 == 1:
                del k_pairs[pair]
            if len(pending) >= 2:
                flush_one()
            if ci == NCHUNK - 1:
                # batches 0..5 are summed by now; finalize them while the
                # last chunks are still in flight
                finalize_batches(0, B_LOC - 2, "a")
            phs = stage_main(ci, kts)
            hts = stage_tanh(ci, phs)
            pending.append((ci, hts))
        while pending:
            flush_one()
        nc.vector.tensor_reduce(
            out=sums8[:, B_LOC - 1 : B_LOC],
            in_=exp_all[:, 16 * (B_LOC - 1) : 16 * B_LOC],
            axis=X,
            op=Add,
        )
        # short tail: only the last two batches
        finalize_batches(B_LOC - 2, B_LOC, "b")


_program_cache = {}


def _get_program() -> bass.Bass:
    if "nc" not in _program_cache:
        _program_cache["nc"] = build_program()
    return _program_cache["nc"]


def kernel(q, k, W1, b1, W2, b2, V, bv=None, **_unused):
    from concourse.bass_utils import run_bass_kernel_spmd

    q = np.ascontiguousarray(np.asarray(q, dtype=np.float32))
    k = np.ascontiguousarray(np.asarray(k, dtype=np.float32))
    W1 = np.ascontiguousarray(np.asarray(W1, dtype=np.float32))
    b1 = np.ascontiguousarray(np.asarray(b1, dtype=np.float32))
    W2 = np.ascontiguousarray(np.asarray(W2, dtype=np.float32))
    b2 = np.ascontiguousarray(np.asarray(b2, dtype=np.float32))
    V = np.ascontiguousarray(np.asarray(V, dtype=np.float32))

    nc = _get_program()
    in_maps = []
    for c in range(NCORES):
        in_maps.append(
            {
                "q": q,
                "k": np.ascontiguousarray(k[c * B_LOC : (c + 1) * B_LOC]),
                "W1": W1,
                "b1": b1,
                "W2": W2,
                "b2": b2,
                "V": V,
            }
        )
    res = run_bass_kernel_spmd(nc, in_maps, list(range(NCORES)))
    outs = [res.results[c]["out"] for c in range(NCORES)]
    return np.concatenate(outs, axis=0).astype(np.float32)


if __name__ == "__main__":
    # smoke: build only
    nc = build_program()
    print("built ok")
